# revision 62
# baseline (speedup 1.0000x reference)
"""DifferentialMultiHeadAttention TRN2 Bass kernel.

Sharding: 2 branches x 16 heads = 32 head-instances, 4 per core (core 0-3:
branch 1, core 4-7: branch 2). Each core computes its heads' attention,
applies its lambda-scaled head-output projection (with the final proj folded
in host-side); the host sums the 8 partial outputs and adds the folded bias.

Performance architecture: the TRN2 PE clock ramps with *continuous* execution
and drops on any idle gap, so the whole program is built as one gap-free PE
instruction stream. Attention blocks (one (chunk, head-pair) at a time) are
software-pipelined: scores for step sp issue ahead of the AV accumulation of
step sp-1, and every exp-latency bubble is plugged by popping a closure from
a fill queue holding the *next* phase's work (QKV projection chains of the
other batch, deferred wo-projections of finished chunks, dummy matmuls as a
last resort).

QKV biases: bq/bk are added to qT/kT during the PSUM->SBUF copy as a
per-partition scalar add (DVE tensor_scalar); bv is folded into the host-side
output bias (softmax rows sum to 1, so out_h = AV/den + bv exactly).
Softmax runs without max-subtraction in a transposed layout (scoresT[s,t]);
the denominator comes from a ones-column appended to V; the per-head
reciprocals are broadcast across partitions with K=1 ones matmuls and applied
with one vector multiply. Matmuls run in bf16 with fp32 PSUM accumulation.

Host-side DMA prep: all inputs are pre-arranged partition-major so every
transfer is large contiguous per-partition rows (fast descriptor gen), and
all input DMAs are issued on one queue in exact consumption order.
"""

import sys

for _p in ("/opt/trn_rl_repo", "/opt/pypackages"):
    if _p not in sys.path:
        sys.path.append(_p)

import numpy as np
import ml_dtypes

MM_NP = ml_dtypes.bfloat16

DIM, H, HD = 1024, 16, 64
B = 2
NDT = DIM // 128   # 8 d-tiles
CH = 512           # token chunk size
NH = 4             # heads per core
NCORES = 8
NWARM = 10         # startup dummy matmuls (cover x DMA + clock ramp)


def build(S=2048):
    """Build the per-core SPMD Bass program for per-batch seq len S."""
    import concourse.bacc as bacc
    import concourse.bass as bass
    import concourse.mybir as mybir
    import concourse.tile as tile

    f32 = mybir.dt.float32
    bf16 = mybir.dt.bfloat16

    T = B * S
    NC = S // CH                 # chunks per batch
    NST = S // 128               # s-tiles per batch

    nc = bacc.Bacc("TRN2", target_bir_lowering=False, debug=False,
                   num_devices=NCORES)

    # host pre-arranges everything partition-major so each DMA is one
    # transfer of large contiguous per-partition rows (fast SWDGE descgen)
    xt = nc.dram_tensor("xt", [128, B * NC, NDT * CH], bf16, kind="ExternalInput")
    wk = nc.dram_tensor("wk", [128, NDT * 256], bf16, kind="ExternalInput")
    wall = nc.dram_tensor("wall", [128, 2 * NDT * 256 + 2 * DIM], bf16,
                          kind="ExternalInput")   # wv | wq | wo
    qkb = nc.dram_tensor("qkb", [128, 4], f32, kind="ExternalInput")
    z = nc.dram_tensor("z", [DIM, T], f32, kind="ExternalOutput")

    with tile.TileContext(nc) as tc:
        with (
            nc.allow_low_precision(
                reason="bf16 operands; PSUM accumulation stays fp32"),
            tc.tile_pool(name="scp", bufs=2, space="PSUM") as scp,
            tc.tile_pool(name="avp", bufs=2, space="PSUM") as avp,
            tc.tile_pool(name="flxA", bufs=1, space="PSUM") as flxA,
            tc.tile_pool(name="flxB", bufs=1, space="PSUM") as flxB,
            tc.tile_pool(name="consts", bufs=1) as consts,
            tc.tile_pool(name="kv", bufs=2) as kv,
            tc.tile_pool(name="xp", bufs=2 * NC) as xp,
            tc.tile_pool(name="work", bufs=4) as work,
            tc.tile_pool(name="outp", bufs=2 * NC + 2) as outp,
        ):
            wk_sb = consts.tile([128, NDT, 256], bf16)
            wall_sb = consts.tile([128, 2 * NDT * 256 + 2 * DIM], bf16)
            qkb_sb = consts.tile([128, 4], f32)
            ones_mm = consts.tile([65, 64], bf16)
            wu = consts.tile([128, CH], bf16, name="wu")

            # startup-critical transfers in consumption order on the gpsimd
            # queue (x-c0 + wk gate the first K chain, qkb its kT copy, wv/wq
            # the V/Q chains); the remaining x chunks stream on the SP queue
            x_blks = {}
            for b in range(B):
                for c in range(NC):
                    x_blks[(b, c)] = xp.tile([128, NDT, CH], bf16, tag="x",
                                             name=f"x{b}_{c}")

            def x_dma(eng, b, c):
                eng.dma_start(out=x_blks[(b, c)],
                              in_=xt.ap()[:, b * NC + c, :].rearrange(
                                  "p (dt t) -> p dt t", dt=NDT))

            # strict FIFO on one queue => bytes land in consumption order
            x_dma(nc.gpsimd, 0, 0)
            nc.gpsimd.dma_start(out=wk_sb, in_=wk.ap().rearrange("p (dt m) -> p dt m", dt=NDT))
            nc.gpsimd.dma_start(out=qkb_sb, in_=qkb.ap())
            wv_sb = wall_sb[:, 0:NDT * 256].rearrange("p (dt m) -> p dt m", dt=NDT)
            wq_sb = wall_sb[:, NDT * 256:2 * NDT * 256].rearrange("p (dt m) -> p dt m", dt=NDT)
            wo_sb = wall_sb[:, 2 * NDT * 256:].rearrange("p (pk n) -> p pk n", pk=2)
            nc.gpsimd.dma_start(out=wall_sb[:, 0:NDT * 256],
                                in_=wall.ap()[:, 0:NDT * 256])
            nc.gpsimd.dma_start(out=wall_sb[:, NDT * 256:2 * NDT * 256],
                                in_=wall.ap()[:, NDT * 256:2 * NDT * 256])
            for c in range(1, NC):
                x_dma(nc.gpsimd, 0, c)
            nc.gpsimd.dma_start(out=wall_sb[:, 2 * NDT * 256:],
                                in_=wall.ap()[:, 2 * NDT * 256:])
            for c in range(NC):
                x_dma(nc.gpsimd, 1, c)

            nc.vector.memset(wu, 0.25)
            nc.vector.memset(ones_mm, 1.0)

            kT, qT, va = {}, {}, {}
            for b in range(B):
                kT[b] = kv.tile([128, 2, S], bf16, tag="kT", name=f"kT{b}")
                qT[b] = kv.tile([128, 2, S], bf16, tag="qT", name=f"qT{b}")
                va[b] = kv.tile([128, NST, NH, 65], bf16, tag="va", name=f"va{b}")
                nc.vector.memset(va[b][:, :, :, 64:65], 1.0)

            # ---- startup warmup: ramp the PE clock while DMAs land ----
            for wi in range(NWARM):
                wp = flxB.tile([128, CH], f32, tag="fill", name=f"warm{wi}")
                nc.tensor.matmul(wp[:], wu[:, 0:128], wu[:], start=True, stop=True)

            # ---- fill queues + drain ----
            opq_bc = []  # bc+mult closures — highest priority (bounded lag)
            opq_m = []   # mandatory (QKV chains) — drained next, FIFO
            opq_f = []   # deferred (wo-projection) — drained when m dry
            # a few fill closures are held back so the final drain has real
            # PE work to cover the last block's epilogue latency
            freserve = [0]
            tail_mode = [False]
            fthrottle = [False]
            nslot = [0]

            def drain(n):
                for _ in range(n):
                    nslot[0] += 1
                    if opq_bc:
                        opq_bc.pop(0)()
                    elif opq_m:
                        opq_m.pop(0)()
                    elif len(opq_f) > freserve[0]:
                        # stretch the remaining fill over the remaining slots
                        # (alternate) so late blocks keep some PE slack too
                        if fthrottle[0] and nslot[0] % 2:
                            continue
                        # keep the OLDEST freserve items as tail stock (their
                        # deps are long resolved); pop the first unreserved
                        opq_f.pop(freserve[0])()
                    # dry queues: no-op — the sc/av stream is ACT-balanced

            def drain_all_m():
                while opq_m:
                    opq_m.pop(0)()

            # ---- QKV projection chain builders (list of closures) ----
            def qk_closures(b, c, wsb, dstT, bidx, pk, pool=None):
                st8 = {}
                pool_ = pool or flxA

                def part(d0):
                    def f():
                        if d0 == 0:
                            st8["ps"] = pool_.tile([128, CH], f32, tag="fill" if pool_ is flxB else "chain",
                                                   name=f"qk{b}_{c}_{bidx}_{pk}")
                        ps = st8["ps"]
                        for dt_i in range(d0, d0 + 4):
                            nc.tensor.matmul(
                                ps[:], wsb[:, dt_i, 128 * pk:128 * pk + 128],
                                x_blks[(b, c)][:, dt_i, :],
                                start=(dt_i == 0), stop=(dt_i == NDT - 1))
                        if d0 == NDT - 4:
                            nc.vector.tensor_scalar(
                                dstT[b][:, pk, c * CH:(c + 1) * CH], ps[:],
                                qkb_sb[:, bidx + pk:bidx + pk + 1], None,
                                mybir.AluOpType.add)
                    return f

                return [part(0), part(4)]

            def v_closures(b, c, tt, pool=None):
                st8 = {}
                pool_ = pool or flxA

                def part(d0):
                    def f():
                        if d0 == 0:
                            st8["ps"] = pool_.tile([128, CH], f32, tag="fill" if pool_ is flxB else "chain",
                                                   name=f"v{b}_{c}_{tt}")
                        ps = st8["ps"]
                        for dt_i in range(d0, d0 + 4):
                            nc.tensor.matmul(
                                ps[:, 0:256], x_blks[(b, c)][:, dt_i, 128 * tt:128 * tt + 128],
                                wv_sb[:, dt_i, :],
                                start=(dt_i == 0), stop=(dt_i == NDT - 1))
                        if d0 == NDT - 4:
                            st = c * 4 + tt
                            nc.vector.tensor_copy(
                                va[b][:, st, :, 0:64],
                                ps[:, 0:256].rearrange("p (h d) -> p h d", h=NH))
                    return f

                return [part(0), part(4)]

            def run_now(cls):
                for f in cls:
                    f()

            # ---- deferred output projection (wo' = lamf*wo@proj folded) ----
            outTs = {}

            def queue_bc(b, c, pk, u, rcp2m):
                outT = outTs[(b, c)]

                def f():
                    bc = flxB.tile([128, CH], f32, tag="fill",
                                   name=f"bc{b}_{c}_{pk}")
                    # disjoint row quadrants (0 and 64) let the PE fuse the
                    # two broadcast matmuls like the score pairs
                    for hh in range(2):
                        r = 64 * hh
                        nc.tensor.matmul(bc[r:r + 64, :],
                                         ones_mm[r:r + 1, :],
                                         rcp2m[r:r + 1, :],
                                         start=True, stop=True)
                    nc.vector.tensor_mul(outT[:, pk, :], u[:], bc[:])
                opq_bc.append(f)

            def queue_zt(b, c):
                tb = b * S + c * CH
                outT = outTs[(b, c)]

                def zt_op(eo):
                    pool_ = flxA if eo % 2 == 0 else flxB

                    def f():
                        zp = pool_.tile([128, CH], f32,
                                        tag="chain" if eo % 2 == 0 else "fill",
                                        name=f"zp{b}_{c}_{eo}")
                        for pk in range(2):
                            nc.tensor.matmul(
                                zp[:], wo_sb[:, pk, eo * 128:(eo + 1) * 128],
                                outT[:, pk, :],
                                start=(pk == 0), stop=(pk == 1))
                        zs = work.tile([128, CH], f32, tag="zs",
                                       name=f"zs{b}_{c}_{eo}")
                        if tail_mode[0] and eo % 2 == 0:
                            # ScalarE is idle at the tail; alternating the
                            # copies across both engines halves the copy tail
                            nc.scalar.copy(zs[:], zp[:])
                        else:
                            nc.vector.tensor_copy(zs[:], zp[:])
                        nc.sync.dma_start(
                            out=z.ap()[eo * 128:(eo + 1) * 128, tb:tb + CH],
                            in_=zs[:])
                    return f

                for eo in range(NDT):
                    opq_f.append(zt_op(eo))

            # ---- attention block: one (batch, chunk, head-pair) ----
            def attn_block(b, c, pk):
                if pk == 0:
                    outTs[(b, c)] = outp.tile([128, 2, CH], bf16, tag="outT",
                                              name=f"outT{b}_{c}")
                avs = [avp.tile([128, CH], f32, tag="av",
                                name=f"av{b}_{c}_{pk}_{hh}") for hh in range(2)]

                def emit_av(ex, st):
                    # both heads' AV for one s-tile (ex holds both heads)
                    for hh in range(2):
                        nc.tensor.matmul(
                            avs[hh][0:65, :], va[b][:, st, 2 * pk + hh, :],
                            ex[:, hh, :],
                            start=(st == 0), stop=(st == NST - 1))

                prev_ex = None
                for sp in range(NST // 2):
                    exs = []
                    for j in range(2):
                        st = 2 * sp + j
                        # group = one s-tile x both heads: its exp can start
                        # after only two score matmuls, maximizing av slack
                        sc = scp.tile([128, 2, CH], f32, tag="sc",
                                      name=f"sc{b}_{c}_{pk}_{sp}_{j}")
                        for hh in range(2):
                            row = 64 * hh
                            nc.tensor.matmul(
                                sc[:, hh, :],
                                kT[b][row:row + 64, pk, st * 128:(st + 1) * 128],
                                qT[b][row:row + 64, pk, c * CH:(c + 1) * CH],
                                start=True, stop=True)
                        ex = work.tile([128, 2, CH], bf16, tag="ex", bufs=4,
                                       name=f"ex{b}_{c}_{pk}_{sp}_{j}")
                        nc.scalar.activation(
                            ex[:], sc[:], mybir.ActivationFunctionType.Exp)
                        exs.append(ex)
                    drain(1)
                    if prev_ex is not None:
                        emit_av(prev_ex[1], 2 * sp - 1)
                    else:
                        drain(2)
                    emit_av(exs[0], 2 * sp)
                    prev_ex = exs
                drain(1)
                emit_av(prev_ex[1], NST - 1)

                # normalization epilogue (DVE; bc matmuls deferred to fill
                # queue). Read each av PSUM bank's two consumers back-to-back
                # so the banks free before the next block's first AV matmuls.
                u = work.tile([128, CH], bf16, tag="u", bufs=4,
                              name=f"u{b}_{c}_{pk}")
                den2 = work.tile([1, 2, CH], f32, tag="den", bufs=2,
                                 name=f"den{b}_{c}_{pk}")
                if tail_mode[0]:
                    # split the last epilogue across ScalarE and DVE so the
                    # final bc/zT drain starts sooner
                    nc.scalar.copy(u[0:64, :], avs[0][0:64, :])
                    nc.scalar.copy(den2[:, 0, :], avs[0][64:65, :])
                else:
                    nc.vector.tensor_copy(u[0:64, :], avs[0][0:64, :])
                    nc.vector.tensor_copy(den2[:, 0, :], avs[0][64:65, :])
                nc.vector.tensor_copy(u[64:128, :], avs[1][0:64, :])
                nc.vector.tensor_copy(den2[:, 1, :], avs[1][64:65, :])
                rcp2 = work.tile([1, 2, CH], f32, tag="rcp", bufs=2,
                                 name=f"rcp{b}_{c}_{pk}")
                nc.vector.reciprocal_approx_fast(rcp2[:], den2[:])
                rcp2m = work.tile([65, CH], bf16, tag="rcpm", bufs=4,
                                  name=f"rcpm{b}_{c}_{pk}")
                nc.vector.tensor_copy(rcp2m[0:1, :], rcp2[:, 0, :])
                nc.vector.tensor_copy(rcp2m[64:65, :], rcp2[:, 1, :])
                queue_bc(b, c, pk, u, rcp2m)
                if pk == 1:
                    queue_zt(b, c)

            # ================= program =================
            # phase A(b0) inline & dense, chunk-major so the PE always has
            # x(c)-dependent work while x(c+1) is still in flight. Chains
            # alternate the two flex PSUM banks so chain n+1's matmuls never
            # wait on chain n's PSUM->SBUF copy (keeps the PE gap-free).
            par = [0]

            def run_alt(mk):
                run_now(mk([flxA, flxB][par[0] & 1]))
                par[0] += 1

            for c in range(NC):
                for pk in range(2):
                    run_alt(lambda p, c=c, pk=pk: qk_closures(0, c, wk_sb, kT, 2, pk, p))
                for tt in range(4):
                    run_alt(lambda p, c=c, tt=tt: v_closures(0, c, tt, p))
                if c == 0:
                    for pk in range(2):
                        run_alt(lambda p, pk=pk: qk_closures(0, 0, wq_sb, qT, 0, pk, p))

            # queue the rest: Q(b0,c1..), then all of batch 1's projections
            for c in range(1, NC):
                for pk in range(2):
                    opq_m += qk_closures(0, c, wq_sb, qT, 0, pk)
            for c in range(NC):
                for pk in range(2):
                    opq_m += qk_closures(1, c, wk_sb, kT, 2, pk)
            for pk in range(2):
                opq_m += qk_closures(1, 0, wq_sb, qT, 0, pk)
            for c in range(NC):
                for tt in range(4):
                    opq_m += v_closures(1, c, tt)
            for c in range(1, NC):
                for pk in range(2):
                    opq_m += qk_closures(1, c, wq_sb, qT, 0, pk)

            # phase B(b0): fill slots consume opq_m (A(b1) work)
            for c in range(NC):
                for pk in range(2):
                    attn_block(0, c, pk)

            # everything batch 1 needs must be emitted before its blocks
            drain_all_m()

            # phase B(b1): fill slots consume opq_f (deferred wo-projections);
            # hold back a few so the final drain covers the last epilogue
            freserve[0] = 10
            fthrottle[0] = True
            for c in range(NC):
                for pk in range(2):
                    if c == NC - 1 and pk == 1:
                        tail_mode[0] = True
                    attn_block(1, c, pk)

            # final drain: reserved stock first (covers the last epilogue's
            # DVE latency), then the last bc, then its wo-projections
            for _ in range(min(freserve[0], len(opq_f))):
                opq_f.pop(0)()
            while opq_bc:
                opq_bc.pop(0)()
            while opq_f:
                opq_f.pop(0)()

    nc.compile()
    return nc


def get_lambda(lambda_param, layer_idx):
    lf = np.clip(float(np.asarray(layer_idx)) * 0.3, 0.0, 5.0)
    offset = 0.6 * np.exp(-lf)
    lam = (1.0 / (1.0 + np.exp(-float(np.asarray(lambda_param).reshape(-1)[0])))
           ) * (1.0 - offset) + 0.2
    return float(np.clip(lam, 0.1, 0.9))


def _pmajor(w):
    """[1024, M] -> partition-major [128, 8*M] (row p holds d=dt*128+p)."""
    m = w.shape[1]
    return np.ascontiguousarray(
        w.reshape(NDT, 128, m).transpose(1, 0, 2).reshape(128, NDT * m))


def prep(inputs, S=2048):
    """Host-side shard prep: returns (in_maps, bias_vec)."""
    NC_ = S // CH
    x = np.asarray(inputs["x"], np.float32)
    T = B * S
    x2 = np.ascontiguousarray(x.reshape(T, DIM))
    # xt[p, cg, dt*CH+tl] = x[cg*CH+tl, dt*128+p]
    xt = np.ascontiguousarray(
        x2.T.reshape(NDT, 128, B * NC_, CH).transpose(1, 2, 0, 3)
        .reshape(128, B * NC_, NDT * CH)).astype(MM_NP)

    lam = get_lambda(inputs["lambda_param"], inputs["layer_idx"])
    pw = np.asarray(inputs["proj_w"], np.float32)
    sc_q = 1.0 / np.sqrt(HD)

    in_maps = []
    for core in range(NCORES):
        br = core // 4 + 1
        lamf = (1.0 - lam) if br == 1 else lam
        hs = slice(4 * (core % 4), 4 * (core % 4) + 4)

        def pick(w, scale=1.0):
            wa = np.asarray(w, np.float32)[:, hs] * scale
            return _pmajor(wa.reshape(DIM, NH * HD))

        # per-partition q/k biases: row r of the pk head-pair tile is
        # head (2pk + r//64), dim r%64
        bq = np.asarray(inputs[f"bq{br}"], np.float32)[hs] * sc_q
        bk = np.asarray(inputs[f"bk{br}"], np.float32)[hs]
        qkb = np.zeros((128, 4), np.float32)
        for pk in range(2):
            qkb[:, 0 + pk] = bq[2 * pk:2 * pk + 2].reshape(128)
            qkb[:, 2 + pk] = bk[2 * pk:2 * pk + 2].reshape(128)

        wo_c = ((np.asarray(inputs[f"wo{br}"], np.float32)[hs] * lamf
                 ).reshape(256, DIM) @ pw)
        wo_pm = np.ascontiguousarray(
            wo_c.reshape(2, 128, DIM).transpose(1, 0, 2).reshape(128, 2 * DIM))
        wall = np.concatenate(
            [pick(inputs[f"wv{br}"]), pick(inputs[f"wq{br}"], sc_q), wo_pm],
            axis=1)
        in_maps.append({
            "xt": xt,
            "wk": pick(inputs[f"wk{br}"]).astype(MM_NP),
            "wall": np.ascontiguousarray(wall).astype(MM_NP),
            "qkb": qkb,
        })

    lam64 = np.float64(lam)
    bias_vec = np.zeros((DIM,), np.float64)
    for br, lamf in ((1, 1.0 - lam64), (2, lam64)):
        bo = np.asarray(inputs[f"bo{br}"], np.float64)
        bv = np.asarray(inputs[f"bv{br}"], np.float64).reshape(H * HD)
        wo_full = np.asarray(inputs[f"wo{br}"], np.float64).reshape(H * HD, DIM)
        bias_vec += lamf * (bo + bv @ wo_full)
    bias_vec = bias_vec @ pw.astype(np.float64) \
        + np.asarray(inputs["proj_b"], np.float64)
    return in_maps, bias_vec


_NC_CACHE = {}


def _get_nc(S=2048):
    if S not in _NC_CACHE:
        _NC_CACHE[S] = build(S)
    return _NC_CACHE[S]


def run(inputs, S=2048, trace=False):
    """Returns (full_output, exec_time_ns_or_None)."""
    from concourse import bass_utils

    nc = _get_nc(S)
    in_maps, bias_vec = prep(inputs, S)
    res = bass_utils.run_bass_kernel_spmd(
        nc, in_maps, core_ids=list(range(NCORES)), trace=trace)
    accT = np.zeros((DIM, B * S), np.float64)
    for c in range(NCORES):
        accT += res.results[c]["z"].astype(np.float64)
    out = (accT.T + bias_vec).reshape(B, S, DIM).astype(np.float32)
    return out, res.exec_time_ns


def kernel(**inputs):
    out, _ = run(inputs, S=2048, trace=False)
    return out
